# revision 12
# baseline (speedup 1.0000x reference)
"""Trainium2 Bass kernel for causal self-attention with RoPE (nn_CausalSelfAttention).

Problem (hardcoded): B=2, S=2048, D=1024, H=16 heads, head_dim=64, fp32,
causal mask, RoPE (rotate-half, base 10000), torch-Linear projections
q = x @ Wq.T, kv = x @ Wkv.T interleaved (k even, v odd output channels).

Sharding: 8 cores = 2 batches x 4 head-groups (4 heads each, as 2 row-packed
pairs). Everything per-core is local; no collectives.

v3 layout choices (all matmul inputs fp16; PSUM accumulation stays f32):
  - x, Wq, Wkv, cos/sin are converted to fp16 on the host and stored
    partition-major so each wave is ONE big DMA (HWDGE fixed overhead is
    ~625ns per DMA -- many small DMAs gated the old wave-0 lead-in).
  - q,k are produced TRANSPOSED per head-pair: (128 partitions = 2 heads x 64
    dims, seq free), fp16 -- directly the scores lhsT/rhs layout.
  - Head dims are permuted on partitions ("paired d-order") so the RoPE
    rotate-half partner is always +16 mod 32 within a 32-partition quadrant,
    implementable with a single DVE stream_shuffle.
  - Scores are computed transposed S^T[k, q] per 128-k-chunk with 2 heads
    row-packed, causally CLIPPED on the q axis (free = 512-lo on diagonal
    chunks) -- fp16 has no small-free matmul penalty.
  - softmax without max-subtraction (scores ~ N(0,1), |s|<~7; exp<=1100 fits
    fp16); exp on ScalarE reads PSUM f32 and writes fp16 probs to SBUF.
  - AV is REORIENTED: out[q(128 partitions), 65 free] per head per q-subchunk
    of 128, lhsT = probs^T chunk [128k, 128q], rhs = [v | ones] fp16
    [128k, 65].  Free size 65 instead of 512 halves AV PE rows; the ones
    column accumulates sum(exp) per q row for free.
  - Normalization + head concat on host from the returned (heads, S, 65)
    tensor (no transpose needed).
  - The whole attention (80 chunks across pair/q-block runs) is ONE flat
    software pipeline: post(c) -> scores(c+1) -> fill -> AV(c), with scores
    crossing run boundaries, so ScalarE (the attention cadence gate at
    ~910ns/chunk) never waits on the in-order PE stream.  Projection/DMA
    work for the next wave is spliced in as <=0.5us closures popped one per
    chunk, with sentinel-based drains so a run's q/k/v inputs are always
    emitted before its first use.
"""

import numpy as np

B, S, D = 2, 2048, 1024
H, HD = 16, 64
NCORES = 8
ROPE_BASE = 10000.0
NKC = D // 128          # contraction chunks for projections (8)
NSC = S // 128          # seq chunks of 128 (16)
NQB = S // 512          # q blocks of 512 (4)

_CACHE = {}


# --------------------------------------------------------------------------
# host-side index maps
# --------------------------------------------------------------------------
def _dperm():
    """Row r (0..63) -> head-dim d, arranged so the rotate-half partner of the
    dim at row r sits at row (r//32)*32 + (r%32+16)%32 (same quadrant)."""
    p = np.empty(64, np.int64)
    for r in range(64):
        quad, i = divmod(r, 32)
        p[r] = 16 * quad + i if i < 16 else 32 + 16 * quad + (i - 16)
    return p


def _rope_tables():
    inv = 1.0 / (ROPE_BASE ** (np.arange(0, HD, 2, dtype=np.float64) / HD))  # (32,)
    t = np.arange(S, dtype=np.float64)
    fr = t[:, None] * inv[None, :]                    # (S, 32)
    return np.cos(fr), np.sin(fr)                     # float64 (S, 32)


# --------------------------------------------------------------------------
# device kernel builder (same NEFF for all 8 cores)
# --------------------------------------------------------------------------
def _build(reps=1, timing=False):
    key = ("nc", reps, timing)
    if key in _CACHE:
        return _CACHE[key]
    import concourse.tile as tile
    from concourse import bacc, mybir

    f32 = mybir.dt.float32
    f16 = mybir.dt.float16
    EXP = mybir.ActivationFunctionType.Exp
    MUL = mybir.AluOpType.mult

    nc = bacc.Bacc("TRN2", target_bir_lowering=False, debug=False)
    kin = "Internal" if timing else "ExternalInput"
    kout = "Internal" if timing else "ExternalOutput"
    # partition-major layouts so wave loads are single 3-D DMAs
    xT = nc.dram_tensor("xT", [128, NKC, S], f16, kind=kin).ap()
    wq = nc.dram_tensor("wq", [128, NKC, 256], f16, kind=kin).ap()
    wk = nc.dram_tensor("wk", [128, NKC, 256], f16, kind=kin).ap()
    wv = nc.dram_tensor("wv", [128, NKC, 256], f16, kind=kin).ap()
    cosT = nc.dram_tensor("cosT", [128, S], f16, kind=kin).ap()
    sinT = nc.dram_tensor("sinT", [128, S], f16, kind=kin).ap()
    tri = nc.dram_tensor("tri", [128, 128], f16, kind=kin).ap()
    vones = nc.dram_tensor("vones", [128, NSC, 4], f16, kind=kin).ap()
    o = nc.dram_tensor("o", [4, S, 65], f32, kind=kout).ap()
    if timing:
        dummy_in = nc.dram_tensor("dummy_in", [1, 64], f32, kind="ExternalInput").ap()
        dummy_out = nc.dram_tensor("dummy_out", [1, 64], f32, kind="ExternalOutput").ap()

    shuf_mask = [(i + 16) % 32 for i in range(32)]

    with tile.TileContext(nc) as tc:
        with (
            tc.tile_pool(name="cst", bufs=1) as cst,
            tc.tile_pool(name="rope", bufs=3) as rope,
            tc.tile_pool(name="ptp", bufs=6) as ptp,
            tc.tile_pool(name="ost", bufs=3) as ost,
            tc.tile_pool(name="pps", bufs=2, space="PSUM") as pps,
            tc.tile_pool(name="scp", bufs=2, space="PSUM") as scp,
            tc.tile_pool(name="ops", bufs=1, space="PSUM") as ops,
        ):
            xT_sbs = [cst.tile([128, NKC, 512], f16, tag=f"xT{i}",
                               name=f"xT_sb{i}") for i in range(4)]
            wq_sb = cst.tile([128, NKC, 256], f16, tag="wq")
            wk_sb = cst.tile([128, NKC, 256], f16, tag="wk")
            wv_sb = cst.tile([128, NKC, 256], f16, tag="wv")
            cos_sb = cst.tile([128, S], f16, tag="cos")
            sin_sb = cst.tile([128, S], f16, tag="sin")
            tri_sb = cst.tile([128, 128], f16, tag="tri")
            qT_sb = cst.tile([128, 2, S], f16, tag="qT")
            kT_sb = cst.tile([128, 2, S], f16, tag="kT")
            vx_sb = cst.tile([128, NSC, 4, 65], f16, tag="vx")

            def proj_qk_mm(dst, w_sb, t, sb, rp, kcs, ps_box):
                """Projection matmuls for a kc range; the last range also
                emits the RoPE ops producing dst[:, t, sl] in fp16."""
                if kcs[0] == 0:
                    ps_box[0] = pps.tile([128, 512], f32, tag="proj",
                                         name=f"ps_{rp}_{id(dst)}_{t}_{sb}")
                ps = ps_box[0]
                for kc in kcs:
                    nc.tensor.matmul(
                        ps[:],
                        w_sb[:, kc, t * 128:(t + 1) * 128],
                        xT_sbs[sb][:, kc, :],
                        start=(kc == 0), stop=(kc == NKC - 1))
                if kcs[-1] == NKC - 1:
                    sl = slice(sb * 512, (sb + 1) * 512)
                    shf = rope.tile([128, 512], f16, tag="shf")
                    nc.vector.stream_shuffle(shf[:], ps[:], shuf_mask)
                    m2 = rope.tile([128, 512], f16, tag="m2")
                    nc.gpsimd.tensor_tensor(m2[:], shf[:], sin_sb[:, sl], MUL)
                    m1 = rope.tile([128, 512], f16, tag="m1")
                    nc.vector.tensor_tensor(m1[:], ps[:], cos_sb[:, sl], MUL)
                    nc.vector.tensor_add(dst[:, t, sl], m1[:], m2[:])

            def proj_v_mm(sc, rp, kcs, ps_box):
                if kcs[0] == 0:
                    ps_box[0] = pps.tile([128, 512], f32, tag="proj",
                                         name=f"psv_{rp}_{sc}")
                psv = ps_box[0]
                for kc in kcs:
                    nc.tensor.matmul(
                        psv[:, 0:256],
                        xT_sbs[sc // 4][:, kc, (sc % 4) * 128:(sc % 4 + 1) * 128],
                        wv_sb[:, kc, :],
                        start=(kc == 0), stop=(kc == NKC - 1))
                if kcs[-1] == NKC - 1:
                    nc.gpsimd.tensor_copy(
                        vx_sb[:, sc, :, 0:64],
                        psv[:, 0:256].rearrange("p (h d) -> p h d", h=4))

            def proj_qk_closures(dst, w_sb, t, sb, rp):
                box = [None]
                return [
                    (lambda ks=ks, b=box: proj_qk_mm(dst, w_sb, t, sb, rp, ks, b))
                    for ks in ([0, 1], [2, 3], [4, 5], [6, 7])
                ]

            def proj_v_closures(sc, rp):
                box = [None]
                return [
                    (lambda ks=ks, b=box: proj_v_mm(sc, rp, ks, b))
                    for ks in ([0, 1, 2, 3], [4, 5, 6, 7])
                ]

            junk_sb = cst.tile([128, 512], f16, tag="junk")

            def pe_warmup(nmm):
                """Zeroed junk matmuls that keep the PE busy while the first
                DMAs land: the cost model's p-state ramp (2-3.7x slower
                cycles until ~3us of continuous PE activity) then burns off
                on throwaway work instead of the critical-path projections."""
                nc.gpsimd.memset(junk_sb[:], 0)
                jp = pps.tile([128, 512], f32, tag="proj", name="junk_ps")
                for _ in range(nmm):
                    nc.tensor.matmul(jp[:], junk_sb[:, 0:128], junk_sb[:],
                                     start=True, stop=True)

            # ---------------- flat attention pipeline ----------------
            # Tile's scheduler is a priority-heap list scheduler (emission
            # index = priority, dependencies enforced globally), so the
            # attention pipeline is emitted FIRST (high priority) and all
            # projection work is emitted AFTER it (low priority): each
            # engine then prefers the attention critical path and fills its
            # dependency stalls with whatever projection work is ready.
            def attn_all(rp):
                chunks = []
                for qb in range(NQB):
                    for pair in range(2):
                        for c in range(4 * qb + 4):
                            chunks.append((pair, qb, c))
                n = len(chunks)
                o_ps_by_run = {}

                def emit_sc(i):
                    pair, qb, c = chunks[i]
                    if c == 0:
                        o_ps_by_run[(pair, qb)] = [
                            ops.tile([128, 4, 65], f32, tag=f"o{h}",
                                     name=f"o_ps{rp}_{pair}_{qb}_{h}")
                            for h in range(2)]
                    s = c - 4 * qb
                    lo = 0 if s < 0 else 128 * s
                    qlo = qb * 512
                    sc_t = scp.tile([128, 2, 512], f32, tag="sc",
                                    name=f"sc_{rp}_{pair}_{qb}_{c}")
                    for h in range(2):
                        nc.tensor.matmul(
                            sc_t[:, h, lo:],
                            kT_sb[h * 64:(h + 1) * 64, pair,
                                  c * 128:(c + 1) * 128],
                            qT_sb[h * 64:(h + 1) * 64, pair,
                                  qlo + lo:qlo + 512],
                            start=True, stop=True)
                    return sc_t

                def emit_post(i, sc_t):
                    pair, qb, c = chunks[i]
                    s = c - 4 * qb
                    lo = 0 if s < 0 else 128 * s
                    pt = ptp.tile([128, 2, 512], f16, tag="pt")
                    nc.scalar.activation(
                        pt[:, :, lo:], sc_t[:, :, lo:], EXP, scale=0.125)
                    if s >= 0:
                        nc.vector.tensor_tensor(
                            pt[:, :, lo:lo + 128],
                            pt[:, :, lo:lo + 128],
                            tri_sb[:].unsqueeze(1).broadcast_to([128, 2, 128]),
                            MUL)
                    return pt

                def emit_av(i, pt):
                    pair, qb, c = chunks[i]
                    s = c - 4 * qb
                    o_ps = o_ps_by_run[(pair, qb)]
                    for j in range(max(0, s), 4):
                        for h in range(2):
                            nc.tensor.matmul(
                                o_ps[h][:, j, :],
                                pt[:, h, j * 128:(j + 1) * 128],
                                vx_sb[:, c, 2 * pair + h, :],
                                start=(c == 0), stop=(c == 4 * qb + j))
                    if c == 4 * qb + 3:  # last chunk of run -> flush
                        qlo = qb * 512
                        o_sb = ost.tile([128, 2, 4, 65], f32, tag="ost")
                        for h in range(2):
                            nc.gpsimd.tensor_copy(o_sb[:, h], o_ps[h][:])
                            nc.sync.dma_start(
                                o[2 * pair + h, qlo:qlo + 512, :]
                                .rearrange("(j p) d -> p j d", j=4),
                                o_sb[:, h])

                sc_t = emit_sc(0)
                for i in range(n):
                    pt = emit_post(i, sc_t)
                    if i + 1 < n:
                        sc_t = emit_sc(i + 1)
                    emit_av(i, pt)

            if timing:
                dpool = cst.tile([1, 64], f32, tag="dumm", name="dumm")
                nc.sync.dma_start(dpool[:], dummy_in)
                nc.sync.dma_start(dummy_out, dpool[:])
            for rp in range(reps):
                # All input DMAs upfront (SP-queue work, no PE cost; ordered
                # so wave-0's critical path lands first).
                nc.sync.dma_start(xT_sbs[0][:], xT[:, :, 0:512])
                nc.sync.dma_start(wq_sb[:], wq[:])
                nc.sync.dma_start(wk_sb[:], wk[:])
                nc.sync.dma_start(cos_sb[:, 0:512], cosT[:, 0:512])
                nc.sync.dma_start(sin_sb[:, 0:512], sinT[:, 0:512])
                nc.sync.dma_start(tri_sb[:], tri)
                nc.sync.dma_start(wv_sb[:], wv[:])
                nc.sync.dma_start(vx_sb[:, :, :, 64], vones)
                for sb in range(1, 4):
                    nc.sync.dma_start(xT_sbs[sb][:],
                                      xT[:, :, sb * 512:(sb + 1) * 512])
                nc.sync.dma_start(cos_sb[:, 512:], cosT[:, 512:])
                nc.sync.dma_start(sin_sb[:, 512:], sinT[:, 512:])
                pe_warmup(7)
                # pair-0 q/k of wave 0 inline: the critical path to the
                # first exp gets the highest projection priority.
                b = [None]
                proj_qk_mm(qT_sb, wq_sb, 0, 0, rp, list(range(NKC)), b)
                b = [None]
                proj_qk_mm(kT_sb, wk_sb, 0, 0, rp, list(range(NKC)), b)
                # the whole attention pipeline, high priority
                attn_all(rp)
                # all remaining projections, low priority, in need-order;
                # the scheduler slots them into attention dependency stalls
                for cl in proj_v_closures(0, rp) + proj_v_closures(1, rp) \
                        + proj_v_closures(2, rp) + proj_v_closures(3, rp):
                    cl()
                for cl in proj_qk_closures(qT_sb, wq_sb, 1, 0, rp) \
                        + proj_qk_closures(kT_sb, wk_sb, 1, 0, rp):
                    cl()
                for sb in range(1, 4):
                    cls = (proj_qk_closures(qT_sb, wq_sb, 0, sb, rp)
                           + proj_qk_closures(kT_sb, wk_sb, 0, sb, rp))
                    for sc in range(4 * sb, 4 * sb + 4):
                        cls += proj_v_closures(sc, rp)
                    cls += (proj_qk_closures(qT_sb, wq_sb, 1, sb, rp)
                            + proj_qk_closures(kT_sb, wk_sb, 1, sb, rp))
                    for cl in cls:
                        cl()

    nc.compile()
    _CACHE[key] = nc
    return nc


# --------------------------------------------------------------------------
# host-side sharding / unsharding
# --------------------------------------------------------------------------
def _make_in_maps(x, Wq, Wkv):
    x = np.asarray(x, np.float32)
    Wq = np.asarray(Wq, np.float32)
    Wkv = np.asarray(Wkv, np.float32)

    dp = _dperm()
    cos32, sin32 = _rope_tables()
    sign = np.where((np.arange(128) % 32) < 16, -1.0, 1.0)
    rows64 = np.concatenate([dp, dp])                       # 128 rows, 2 heads
    cosT = cos32[:, rows64 % 32].T.astype(np.float16)       # (128, S)
    sinT = (sin32[:, rows64 % 32].T * sign[:, None]).astype(np.float16)
    tri = (np.arange(128)[:, None] <= np.arange(128)[None, :]).astype(np.float16)

    # x[b].T is (D, S) = (NKC*128, S); partition-major: (128, NKC, S)
    xT_b = [np.ascontiguousarray(
                x[b].T.reshape(NKC, 128, S).transpose(1, 0, 2)
            ).astype(np.float16) for b in range(B)]

    in_maps = []
    for c in range(NCORES):
        b, g = divmod(c, 4)
        heads = [4 * g + hh for hh in range(4)]
        qrows = np.concatenate([h * 64 + dp for h in heads])
        krows = np.concatenate([h * 128 + 2 * dp for h in heads])
        vrows = np.concatenate([h * 128 + 2 * np.arange(64) + 1 for h in heads])

        def wmap(Wrows):  # (256, D) -> partition-major (128, NKC, 256) fp16
            wT = Wrows.T.reshape(NKC, 128, 256).transpose(1, 0, 2)
            return np.ascontiguousarray(wT).astype(np.float16)

        in_maps.append({
            "xT": xT_b[b],
            "wq": wmap(Wq[qrows, :]),
            "wk": wmap(Wkv[krows, :]),
            "wv": wmap(Wkv[vrows, :]),
            "cosT": cosT, "sinT": sinT, "tri": tri,
            "vones": np.ones((128, NSC, 4), np.float16),
        })
    return in_maps


def _assemble(results):
    out = np.empty((B, S, D), np.float32)
    for c in range(NCORES):
        b, g = divmod(c, 4)
        oc = results[c]["o"]                        # (4, S, 65)
        att = oc[:, :, :64] / oc[:, :, 64:65]       # (4, S, 64)
        for hh in range(4):
            head = 4 * g + hh
            out[b, :, head * 64:(head + 1) * 64] = att[hh]
    return out


def kernel(x, Wq, Wkv, mask=None):
    from concourse.bass_utils import run_bass_kernel_spmd

    nc = _build()
    in_maps = _make_in_maps(x, Wq, Wkv)
    res = run_bass_kernel_spmd(nc, in_maps, core_ids=list(range(NCORES)))
    return _assemble(res.results)


# revision 15
# speedup vs baseline: 1.1631x; 1.1631x over previous
"""Trainium2 Bass kernel for causal self-attention with RoPE (nn_CausalSelfAttention).

Problem (hardcoded): B=2, S=2048, D=1024, H=16 heads, head_dim=64, fp32,
causal mask, RoPE (rotate-half, base 10000), torch-Linear projections
q = x @ Wq.T, kv = x @ Wkv.T interleaved (k even, v odd output channels).

Sharding: 8 cores = 2 batches x 4 head-groups (4 heads each, as 2 row-packed
pairs). Everything per-core is local; no collectives.

v3 layout choices (all matmul inputs fp16; PSUM accumulation stays f32):
  - x, Wq, Wkv, cos/sin are converted to fp16 on the host and stored
    partition-major so each wave is ONE big DMA (HWDGE fixed overhead is
    ~625ns per DMA -- many small DMAs gated the old wave-0 lead-in).
  - q,k are produced TRANSPOSED per head-pair: (128 partitions = 2 heads x 64
    dims, seq free), fp16 -- directly the scores lhsT/rhs layout.
  - Head dims are permuted on partitions ("paired d-order") so the RoPE
    rotate-half partner is always +16 mod 32 within a 32-partition quadrant,
    implementable with a single DVE stream_shuffle.
  - Scores are computed transposed S^T[k, q] per 128-k-chunk with 2 heads
    row-packed, causally CLIPPED on the q axis (free = 512-lo on diagonal
    chunks) -- fp16 has no small-free matmul penalty.
  - softmax without max-subtraction (scores ~ N(0,1), |s|<~7; exp<=1100 fits
    fp16); exp on ScalarE reads PSUM f32 and writes fp16 probs to SBUF.
  - AV is REORIENTED: out[q(128 partitions), 65 free] per head per q-subchunk
    of 128, lhsT = probs^T chunk [128k, 128q], rhs = [v | ones] fp16
    [128k, 65].  Free size 65 instead of 512 halves AV PE rows; the ones
    column accumulates sum(exp) per q row for free.
  - Normalization + head concat on host from the returned (heads, S, 65)
    tensor (no transpose needed).
  - The whole attention (80 chunks across pair/q-block runs) is ONE flat
    software pipeline: post(c) -> scores(c+1) -> fill -> AV(c), with scores
    crossing run boundaries, so ScalarE (the attention cadence gate at
    ~910ns/chunk) never waits on the in-order PE stream.  Projection/DMA
    work for the next wave is spliced in as <=0.5us closures popped one per
    chunk, with sentinel-based drains so a run's q/k/v inputs are always
    emitted before its first use.
"""

import numpy as np

B, S, D = 2, 2048, 1024
H, HD = 16, 64
NCORES = 8
ROPE_BASE = 10000.0
NKC = D // 128          # contraction chunks for projections (8)
NSC = S // 128          # seq chunks of 128 (16)
NQB = S // 512          # q blocks of 512 (4)

_CACHE = {}


# --------------------------------------------------------------------------
# host-side index maps
# --------------------------------------------------------------------------
def _dperm():
    """Row r (0..63) -> head-dim d, arranged so the rotate-half partner of the
    dim at row r sits at row (r//32)*32 + (r%32+16)%32 (same quadrant)."""
    p = np.empty(64, np.int64)
    for r in range(64):
        quad, i = divmod(r, 32)
        p[r] = 16 * quad + i if i < 16 else 32 + 16 * quad + (i - 16)
    return p


def _rope_tables():
    inv = 1.0 / (ROPE_BASE ** (np.arange(0, HD, 2, dtype=np.float64) / HD))  # (32,)
    t = np.arange(S, dtype=np.float64)
    fr = t[:, None] * inv[None, :]                    # (S, 32)
    return np.cos(fr), np.sin(fr)                     # float64 (S, 32)


# --------------------------------------------------------------------------
# device kernel builder (same NEFF for all 8 cores)
# --------------------------------------------------------------------------
def _build(reps=1, timing=False):
    key = ("nc", reps, timing)
    if key in _CACHE:
        return _CACHE[key]
    import concourse.tile as tile
    from concourse import bacc, mybir

    f32 = mybir.dt.float32
    f16 = mybir.dt.float16
    EXP = mybir.ActivationFunctionType.Exp
    MUL = mybir.AluOpType.mult

    nc = bacc.Bacc("TRN2", target_bir_lowering=False, debug=False)
    kin = "Internal" if timing else "ExternalInput"
    kout = "Internal" if timing else "ExternalOutput"
    # partition-major layouts so wave loads are single 3-D DMAs
    xT = nc.dram_tensor("xT", [128, NKC, S], f16, kind=kin).ap()
    wq = nc.dram_tensor("wq", [128, NKC, 256], f16, kind=kin).ap()
    wk = nc.dram_tensor("wk", [128, NKC, 256], f16, kind=kin).ap()
    wv = nc.dram_tensor("wv", [128, NKC, 256], f16, kind=kin).ap()
    cosT = nc.dram_tensor("cosT", [128, S], f16, kind=kin).ap()
    sinT = nc.dram_tensor("sinT", [128, S], f16, kind=kin).ap()
    tri = nc.dram_tensor("tri", [128, 128], f16, kind=kin).ap()
    vones = nc.dram_tensor("vones", [128, NSC, 4], f16, kind=kin).ap()
    o = nc.dram_tensor("o", [4, S, 65], f32, kind=kout).ap()
    if timing:
        dummy_in = nc.dram_tensor("dummy_in", [1, 64], f32, kind="ExternalInput").ap()
        dummy_out = nc.dram_tensor("dummy_out", [1, 64], f32, kind="ExternalOutput").ap()

    shuf_mask = [(i + 16) % 32 for i in range(32)]

    with tile.TileContext(nc) as tc:
        with (
            tc.tile_pool(name="cst", bufs=1) as cst,
            tc.tile_pool(name="rope", bufs=3) as rope,
            tc.tile_pool(name="ptp", bufs=6) as ptp,
            tc.tile_pool(name="ost", bufs=3) as ost,
            tc.tile_pool(name="pps", bufs=2, space="PSUM") as pps,
            tc.tile_pool(name="scp", bufs=2, space="PSUM") as scp,
            tc.tile_pool(name="ops", bufs=1, space="PSUM") as ops,
        ):
            xT_sbs = [cst.tile([128, NKC, 512], f16, tag=f"xT{i}",
                               name=f"xT_sb{i}") for i in range(4)]
            wq_sb = cst.tile([128, NKC, 256], f16, tag="wq")
            wk_sb = cst.tile([128, NKC, 256], f16, tag="wk")
            wv_sb = cst.tile([128, NKC, 256], f16, tag="wv")
            cos_sb = cst.tile([128, S], f16, tag="cos")
            sin_sb = cst.tile([128, S], f16, tag="sin")
            tri_sb = cst.tile([128, 128], f16, tag="tri")
            qT_sb = cst.tile([128, 2, S], f16, tag="qT")
            kT_sb = cst.tile([128, 2, S], f16, tag="kT")
            vx_sb = cst.tile([128, NSC, 4, 65], f16, tag="vx")

            def proj_qk_mm(dst, w_sb, t, sb, rp, kcs, ps_box):
                """Projection matmuls for a kc range; the last range also
                emits the RoPE ops producing dst[:, t, sl] in fp16."""
                if kcs[0] == 0:
                    ps_box[0] = pps.tile([128, 512], f32, tag="proj",
                                         name=f"ps_{rp}_{id(dst)}_{t}_{sb}")
                ps = ps_box[0]
                for kc in kcs:
                    nc.tensor.matmul(
                        ps[:],
                        w_sb[:, kc, t * 128:(t + 1) * 128],
                        xT_sbs[sb][:, kc, :],
                        start=(kc == 0), stop=(kc == NKC - 1))
                if kcs[-1] == NKC - 1:
                    sl = slice(sb * 512, (sb + 1) * 512)
                    shf = rope.tile([128, 512], f16, tag="shf")
                    nc.vector.stream_shuffle(shf[:], ps[:], shuf_mask)
                    m2 = rope.tile([128, 512], f16, tag="m2")
                    nc.gpsimd.tensor_tensor(m2[:], shf[:], sin_sb[:, sl], MUL)
                    m1 = rope.tile([128, 512], f16, tag="m1")
                    nc.vector.tensor_tensor(m1[:], ps[:], cos_sb[:, sl], MUL)
                    nc.vector.tensor_add(dst[:, t, sl], m1[:], m2[:])

            def proj_v_mm(sc, rp, kcs, ps_box):
                if kcs[0] == 0:
                    ps_box[0] = pps.tile([128, 512], f32, tag="proj",
                                         name=f"psv_{rp}_{sc}")
                psv = ps_box[0]
                for kc in kcs:
                    nc.tensor.matmul(
                        psv[:, 0:256],
                        xT_sbs[sc // 4][:, kc, (sc % 4) * 128:(sc % 4 + 1) * 128],
                        wv_sb[:, kc, :],
                        start=(kc == 0), stop=(kc == NKC - 1))
                if kcs[-1] == NKC - 1:
                    nc.gpsimd.tensor_copy(
                        vx_sb[:, sc, :, 0:64],
                        psv[:, 0:256].rearrange("p (h d) -> p h d", h=4))

            def proj_qk_closures(dst, w_sb, t, sb, rp):
                box = [None]
                return [
                    (lambda ks=ks, b=box: proj_qk_mm(dst, w_sb, t, sb, rp, ks, b))
                    for ks in ([0, 1], [2, 3], [4, 5], [6, 7])
                ]

            def proj_v_closures(sc, rp):
                box = [None]
                return [
                    (lambda ks=ks, b=box: proj_v_mm(sc, rp, ks, b))
                    for ks in ([0, 1, 2, 3], [4, 5, 6, 7])
                ]

            junk_sb = cst.tile([128, 512], f16, tag="junk")

            def pe_warmup(nmm):
                """Zeroed junk matmuls that keep the PE busy while the first
                DMAs land: the cost model's p-state ramp (2-3.7x slower
                cycles until ~3us of continuous PE activity) then burns off
                on throwaway work instead of the critical-path projections."""
                nc.gpsimd.memset(junk_sb[:], 0)
                jp = pps.tile([128, 512], f32, tag="proj", name="junk_ps")
                for _ in range(nmm):
                    nc.tensor.matmul(jp[:], junk_sb[:, 0:128], junk_sb[:],
                                     start=True, stop=True)

            # ---------------- fill queue with drain sentinels ----------------
            fill_q = []

            def fill(n=1):
                done = 0
                while fill_q and done < n:
                    item = fill_q[0]
                    if isinstance(item, str):
                        break  # sentinels are only crossed by drain_until
                    fill_q.pop(0)()
                    done += 1

            def drain_until(marker):
                while fill_q:
                    item = fill_q.pop(0)
                    if isinstance(item, str):
                        if item == marker:
                            return
                        continue
                    item()

            def queue_wave(sb, rp):
                """Queue seq-block-sb projections as <=0.5us closures.
                Layout: [q-t0, k-t0, v x4] <marker A{sb}> [q-t1, k-t1]
                <marker B{sb}>.  The x DMA is issued immediately (it costs no
                PE time; the 4 x buffers make early issue hazard-free)."""
                nc.sync.dma_start(xT_sbs[sb][:],
                                  xT[:, :, sb * 512:(sb + 1) * 512])
                fill_q.extend(proj_qk_closures(qT_sb, wq_sb, 0, sb, rp))
                fill_q.extend(proj_qk_closures(kT_sb, wk_sb, 0, sb, rp))
                for sc in range(4 * sb, 4 * sb + 4):
                    fill_q.extend(proj_v_closures(sc, rp))
                fill_q.append(f"A{sb}")
                fill_q.extend(proj_qk_closures(qT_sb, wq_sb, 1, sb, rp))
                fill_q.extend(proj_qk_closures(kT_sb, wk_sb, 1, sb, rp))
                fill_q.append(f"B{sb}")

            # ---------------- flat attention pipeline ----------------
            def attn_all(rp):
                chunks = []
                for qb in range(NQB):
                    for pair in range(2):
                        for c in range(4 * qb + 4):
                            chunks.append((pair, qb, c))
                n = len(chunks)
                o_ps_by_run = {}

                def emit_sc(i):
                    pair, qb, c = chunks[i]
                    if c == 0:
                        # run boundary: everything this run reads must have
                        # been emitted already (the scheduler leans heavily
                        # on emission order within each engine)
                        drain_until(f"A{qb}" if pair == 0 else f"B{qb}")
                        if qb > 0 and pair == 0 and qb + 1 < NQB:
                            queue_wave(qb + 1, rp)
                        o_ps_by_run[(pair, qb)] = [
                            ops.tile([128, 4, 65], f32, tag=f"o{h}",
                                     name=f"o_ps{rp}_{pair}_{qb}_{h}")
                            for h in range(2)]
                    s = c - 4 * qb
                    lo = 0 if s < 0 else 128 * s
                    qlo = qb * 512
                    sc_t = scp.tile([128, 2, 512], f32, tag="sc",
                                    name=f"sc_{rp}_{pair}_{qb}_{c}")
                    for h in range(2):
                        nc.tensor.matmul(
                            sc_t[:, h, lo:],
                            kT_sb[h * 64:(h + 1) * 64, pair,
                                  c * 128:(c + 1) * 128],
                            qT_sb[h * 64:(h + 1) * 64, pair,
                                  qlo + lo:qlo + 512],
                            start=True, stop=True)
                    return sc_t

                def emit_post(i, sc_t):
                    pair, qb, c = chunks[i]
                    s = c - 4 * qb
                    lo = 0 if s < 0 else 128 * s
                    pt = ptp.tile([128, 2, 512], f16, tag="pt")
                    nc.scalar.activation(
                        pt[:, :, lo:], sc_t[:, :, lo:], EXP, scale=0.125)
                    if s >= 0:
                        nc.vector.tensor_tensor(
                            pt[:, :, lo:lo + 128],
                            pt[:, :, lo:lo + 128],
                            tri_sb[:].unsqueeze(1).broadcast_to([128, 2, 128]),
                            MUL)
                    return pt

                def emit_av(i, pt):
                    pair, qb, c = chunks[i]
                    s = c - 4 * qb
                    o_ps = o_ps_by_run[(pair, qb)]
                    for j in range(max(0, s), 4):
                        for h in range(2):
                            nc.tensor.matmul(
                                o_ps[h][:, j, :],
                                pt[:, h, j * 128:(j + 1) * 128],
                                vx_sb[:, c, 2 * pair + h, :],
                                start=(c == 0), stop=(c == 4 * qb + j))
                    if c == 4 * qb + 3:  # last chunk of run -> flush
                        qlo = qb * 512
                        o_sb = ost.tile([128, 2, 4, 65], f32, tag="ost")
                        for h in range(2):
                            nc.gpsimd.tensor_copy(o_sb[:, h], o_ps[h][:])
                            nc.sync.dma_start(
                                o[2 * pair + h, qlo:qlo + 512, :]
                                .rearrange("(j p) d -> p j d", j=4),
                                o_sb[:, h])

                sc_t = emit_sc(0)
                for i in range(n):
                    pt = emit_post(i, sc_t)
                    if i + 1 < n:
                        sc_t = emit_sc(i + 1)
                    # first run's fill must keep pace with its own v chunks
                    # (2 closures per chunk); elsewhere 1 per chunk matches
                    # the ~400ns PE slack under the ScalarE exp cadence
                    fill(2 if i < 4 else 1)
                    emit_av(i, pt)
                drain_until("nonexistent")  # safety: empty the queue

            if timing:
                dpool = cst.tile([1, 64], f32, tag="dumm", name="dumm")
                nc.sync.dma_start(dpool[:], dummy_in)
                nc.sync.dma_start(dummy_out, dpool[:])
            for rp in range(reps):
                # Wave 0, minimal critical path to the first exp:
                nc.sync.dma_start(xT_sbs[0][:], xT[:, :, 0:512])
                nc.sync.dma_start(wq_sb[:], wq[:])
                nc.sync.dma_start(wk_sb[:], wk[:])
                nc.sync.dma_start(cos_sb[:, 0:512], cosT[:, 0:512])
                nc.sync.dma_start(sin_sb[:, 0:512], sinT[:, 0:512])
                nc.sync.dma_start(tri_sb[:], tri)
                nc.sync.dma_start(wv_sb[:], wv[:])
                nc.sync.dma_start(vx_sb[:, :, :, 64], vones)
                nc.sync.dma_start(cos_sb[:, 512:], cosT[:, 512:])
                nc.sync.dma_start(sin_sb[:, 512:], sinT[:, 512:])
                pe_warmup(7)
                # pair-0 q/k inline (attention can then start); v chunks 0..3
                # go first in the fill queue, popped 2-per-chunk during run
                # (0,0) just in time for each chunk's AV.
                b = [None]
                proj_qk_mm(qT_sb, wq_sb, 0, 0, rp, list(range(NKC)), b)
                b = [None]
                proj_qk_mm(kT_sb, wk_sb, 0, 0, rp, list(range(NKC)), b)
                for sc in range(4):
                    fill_q.extend(proj_v_closures(sc, rp))
                # pair-1 q/k of wave 0 go through the fill queue (spliced into
                # attention run (0,0)); then wave 1.
                fill_q.append("A0")
                fill_q.extend(proj_qk_closures(qT_sb, wq_sb, 1, 0, rp))
                fill_q.extend(proj_qk_closures(kT_sb, wk_sb, 1, 0, rp))
                fill_q.append("B0")
                queue_wave(1, rp)
                attn_all(rp)

    nc.compile()
    _CACHE[key] = nc
    return nc


# --------------------------------------------------------------------------
# host-side sharding / unsharding
# --------------------------------------------------------------------------
def _make_in_maps(x, Wq, Wkv):
    x = np.asarray(x, np.float32)
    Wq = np.asarray(Wq, np.float32)
    Wkv = np.asarray(Wkv, np.float32)

    dp = _dperm()
    cos32, sin32 = _rope_tables()
    sign = np.where((np.arange(128) % 32) < 16, -1.0, 1.0)
    rows64 = np.concatenate([dp, dp])                       # 128 rows, 2 heads
    cosT = cos32[:, rows64 % 32].T.astype(np.float16)       # (128, S)
    sinT = (sin32[:, rows64 % 32].T * sign[:, None]).astype(np.float16)
    tri = (np.arange(128)[:, None] <= np.arange(128)[None, :]).astype(np.float16)

    # x[b].T is (D, S) = (NKC*128, S); partition-major: (128, NKC, S)
    xT_b = [np.ascontiguousarray(
                x[b].T.reshape(NKC, 128, S).transpose(1, 0, 2)
            ).astype(np.float16) for b in range(B)]

    in_maps = []
    for c in range(NCORES):
        b, g = divmod(c, 4)
        heads = [4 * g + hh for hh in range(4)]
        qrows = np.concatenate([h * 64 + dp for h in heads])
        krows = np.concatenate([h * 128 + 2 * dp for h in heads])
        vrows = np.concatenate([h * 128 + 2 * np.arange(64) + 1 for h in heads])

        def wmap(Wrows):  # (256, D) -> partition-major (128, NKC, 256) fp16
            wT = Wrows.T.reshape(NKC, 128, 256).transpose(1, 0, 2)
            return np.ascontiguousarray(wT).astype(np.float16)

        in_maps.append({
            "xT": xT_b[b],
            "wq": wmap(Wq[qrows, :]),
            "wk": wmap(Wkv[krows, :]),
            "wv": wmap(Wkv[vrows, :]),
            "cosT": cosT, "sinT": sinT, "tri": tri,
            "vones": np.ones((128, NSC, 4), np.float16),
        })
    return in_maps


def _assemble(results):
    out = np.empty((B, S, D), np.float32)
    for c in range(NCORES):
        b, g = divmod(c, 4)
        oc = results[c]["o"]                        # (4, S, 65)
        att = oc[:, :, :64] / oc[:, :, 64:65]       # (4, S, 64)
        for hh in range(4):
            head = 4 * g + hh
            out[b, :, head * 64:(head + 1) * 64] = att[hh]
    return out


def kernel(x, Wq, Wkv, mask=None):
    from concourse.bass_utils import run_bass_kernel_spmd

    nc = _build()
    in_maps = _make_in_maps(x, Wq, Wkv)
    res = run_bass_kernel_spmd(nc, in_maps, core_ids=list(range(NCORES)))
    return _assemble(res.results)


# revision 21
# speedup vs baseline: 1.1984x; 1.0303x over previous
"""Trainium2 Bass kernel for causal self-attention with RoPE (nn_CausalSelfAttention).

Problem (hardcoded): B=2, S=2048, D=1024, H=16 heads, head_dim=64, fp32,
causal mask, RoPE (rotate-half, base 10000), torch-Linear projections
q = x @ Wq.T, kv = x @ Wkv.T interleaved (k even, v odd output channels).

Sharding: 8 cores = 2 batches x 4 head-groups (4 heads each, as 2 row-packed
pairs). Everything per-core is local; no collectives.

v3 layout choices (all matmul inputs fp16; PSUM accumulation stays f32):
  - x, Wq, Wkv, cos/sin are converted to fp16 on the host and stored
    partition-major so each wave is ONE big DMA (HWDGE fixed overhead is
    ~625ns per DMA -- many small DMAs gated the old wave-0 lead-in).
  - q,k are produced TRANSPOSED per head-pair: (128 partitions = 2 heads x 64
    dims, seq free), fp16 -- directly the scores lhsT/rhs layout.
  - Head dims are permuted on partitions ("paired d-order") so the RoPE
    rotate-half partner is always +16 mod 32 within a 32-partition quadrant,
    implementable with a single DVE stream_shuffle.
  - Scores are computed transposed S^T[k, q] per 128-k-chunk with 2 heads
    row-packed, causally CLIPPED on the q axis (free = 512-lo on diagonal
    chunks) -- fp16 has no small-free matmul penalty.
  - softmax without max-subtraction (scores ~ N(0,1), |s|<~7; exp<=1100 fits
    fp16); exp on ScalarE reads PSUM f32 and writes fp16 probs to SBUF.
  - AV is REORIENTED: out[q(128 partitions), 65 free] per head per q-subchunk
    of 128, lhsT = probs^T chunk [128k, 128q], rhs = [v | ones] fp16
    [128k, 65].  Free size 65 instead of 512 halves AV PE rows; the ones
    column accumulates sum(exp) per q row for free.
  - Normalization + head concat on host from the returned (heads, S, 65)
    tensor (no transpose needed).
  - The whole attention (80 chunks across pair/q-block runs) is ONE flat
    software pipeline: post(c) -> scores(c+1) -> fill -> AV(c), with scores
    crossing run boundaries, so ScalarE (the attention cadence gate at
    ~910ns/chunk) never waits on the in-order PE stream.  Projection/DMA
    work for the next wave is spliced in as <=0.5us closures popped one per
    chunk, with sentinel-based drains so a run's q/k/v inputs are always
    emitted before its first use.
"""

import numpy as np

B, S, D = 2, 2048, 1024
H, HD = 16, 64
NCORES = 8
ROPE_BASE = 10000.0
NKC = D // 128          # contraction chunks for projections (8)
NSC = S // 128          # seq chunks of 128 (16)
NQB = S // 512          # q blocks of 512 (4)

_CACHE = {}


# --------------------------------------------------------------------------
# host-side index maps
# --------------------------------------------------------------------------
def _dperm():
    """Row r (0..63) -> head-dim d, arranged so the rotate-half partner of the
    dim at row r sits at row (r//32)*32 + (r%32+16)%32 (same quadrant)."""
    p = np.empty(64, np.int64)
    for r in range(64):
        quad, i = divmod(r, 32)
        p[r] = 16 * quad + i if i < 16 else 32 + 16 * quad + (i - 16)
    return p


def _rope_tables():
    inv = 1.0 / (ROPE_BASE ** (np.arange(0, HD, 2, dtype=np.float64) / HD))  # (32,)
    t = np.arange(S, dtype=np.float64)
    fr = t[:, None] * inv[None, :]                    # (S, 32)
    return np.cos(fr), np.sin(fr)                     # float64 (S, 32)


# --------------------------------------------------------------------------
# device kernel builder (same NEFF for all 8 cores)
# --------------------------------------------------------------------------
def _build(reps=1, timing=False):
    key = ("nc", reps, timing)
    if key in _CACHE:
        return _CACHE[key]
    import concourse.tile as tile
    from concourse import bacc, mybir

    f32 = mybir.dt.float32
    f16 = mybir.dt.float16
    EXP = mybir.ActivationFunctionType.Exp
    MUL = mybir.AluOpType.mult

    nc = bacc.Bacc("TRN2", target_bir_lowering=False, debug=False)
    kin = "Internal" if timing else "ExternalInput"
    kout = "Internal" if timing else "ExternalOutput"
    # partition-major layouts so wave loads are single 3-D DMAs
    xT = nc.dram_tensor("xT", [128, NKC, S], f16, kind=kin).ap()
    wq = nc.dram_tensor("wq", [128, NKC, 256], f16, kind=kin).ap()
    wk = nc.dram_tensor("wk", [128, NKC, 256], f16, kind=kin).ap()
    wv = nc.dram_tensor("wv", [128, NKC, 256], f16, kind=kin).ap()
    cosT = nc.dram_tensor("cosT", [128, S], f16, kind=kin).ap()
    sinT = nc.dram_tensor("sinT", [128, S], f16, kind=kin).ap()
    tri = nc.dram_tensor("tri", [128, 128], f16, kind=kin).ap()
    vones = nc.dram_tensor("vones", [128, NSC, 4], f16, kind=kin).ap()
    o = nc.dram_tensor("o", [4, S, 65], f32, kind=kout).ap()
    if timing:
        dummy_in = nc.dram_tensor("dummy_in", [1, 64], f32, kind="ExternalInput").ap()
        dummy_out = nc.dram_tensor("dummy_out", [1, 64], f32, kind="ExternalOutput").ap()

    shuf_mask = [(i + 16) % 32 for i in range(32)]

    with tile.TileContext(nc) as tc:
        with (
            tc.tile_pool(name="cst", bufs=1) as cst,
            tc.tile_pool(name="rope", bufs=3) as rope,
            tc.tile_pool(name="ptp", bufs=6) as ptp,
            tc.tile_pool(name="ost", bufs=3) as ost,
            tc.tile_pool(name="pps", bufs=2, space="PSUM") as pps,
            tc.tile_pool(name="scp", bufs=2, space="PSUM") as scp,
            tc.tile_pool(name="ops", bufs=1, space="PSUM") as ops,
        ):
            xT_sbs = [cst.tile([128, NKC, 512], f16, tag=f"xT{i}",
                               name=f"xT_sb{i}") for i in range(4)]
            wq_sb = cst.tile([128, NKC, 256], f16, tag="wq")
            wk_sb = cst.tile([128, NKC, 256], f16, tag="wk")
            wv_sb = cst.tile([128, NKC, 256], f16, tag="wv")
            cos_sb = cst.tile([128, S], f16, tag="cos")
            sin_sb = cst.tile([128, S], f16, tag="sin")
            tri_sb = cst.tile([128, 128], f16, tag="tri")
            qT_sb = cst.tile([128, 2, S], f16, tag="qT")
            kT_sb = cst.tile([128, 2, S], f16, tag="kT")
            vx_sb = cst.tile([128, NSC, 4, 65], f16, tag="vx")

            def proj_qk_mm(dst, w_sb, t, sb, rp, kcs, ps_box):
                """Projection matmuls for a kc range; the last range also
                emits the RoPE ops producing dst[:, t, sl] in fp16."""
                if kcs[0] == 0:
                    ps_box[0] = pps.tile([128, 512], f32, tag="proj",
                                         name=f"ps_{rp}_{id(dst)}_{t}_{sb}")
                ps = ps_box[0]
                for kc in kcs:
                    nc.tensor.matmul(
                        ps[:],
                        w_sb[:, kc, t * 128:(t + 1) * 128],
                        xT_sbs[sb][:, kc, :],
                        start=(kc == 0), stop=(kc == NKC - 1))
                if kcs[-1] == NKC - 1:
                    sl = slice(sb * 512, (sb + 1) * 512)
                    shf = rope.tile([128, 512], f16, tag="shf")
                    nc.vector.stream_shuffle(shf[:], ps[:], shuf_mask)
                    m2 = rope.tile([128, 512], f16, tag="m2")
                    nc.gpsimd.tensor_tensor(m2[:], shf[:], sin_sb[:, sl], MUL)
                    m1 = rope.tile([128, 512], f16, tag="m1")
                    nc.vector.tensor_tensor(m1[:], ps[:], cos_sb[:, sl], MUL)
                    nc.vector.tensor_add(dst[:, t, sl], m1[:], m2[:])

            def proj_v_mm(sc, rp, kcs, ps_box):
                if kcs[0] == 0:
                    ps_box[0] = pps.tile([128, 512], f32, tag="proj",
                                         name=f"psv_{rp}_{sc}")
                psv = ps_box[0]
                for kc in kcs:
                    nc.tensor.matmul(
                        psv[:, 0:256],
                        xT_sbs[sc // 4][:, kc, (sc % 4) * 128:(sc % 4 + 1) * 128],
                        wv_sb[:, kc, :],
                        start=(kc == 0), stop=(kc == NKC - 1))
                if kcs[-1] == NKC - 1:
                    nc.gpsimd.tensor_copy(
                        vx_sb[:, sc, :, 0:64],
                        psv[:, 0:256].rearrange("p (h d) -> p h d", h=4))

            def proj_qk_closures(dst, w_sb, t, sb, rp):
                box = [None]
                return [
                    (lambda ks=ks, b=box: proj_qk_mm(dst, w_sb, t, sb, rp, ks, b))
                    for ks in ([0, 1], [2, 3], [4, 5], [6, 7])
                ]

            def proj_v_closures(sc, rp):
                box = [None]
                return [
                    (lambda ks=ks, b=box: proj_v_mm(sc, rp, ks, b))
                    for ks in ([0, 1, 2, 3], [4, 5, 6, 7])
                ]

            junk_sb = cst.tile([128, 512], f16, tag="junk")

            def pe_warmup(nmm):
                """Zeroed junk matmuls that keep the PE busy while the first
                DMAs land: the cost model's p-state ramp (2-3.7x slower
                cycles until ~3us of continuous PE activity) then burns off
                on throwaway work instead of the critical-path projections."""
                nc.gpsimd.memset(junk_sb[:], 0)
                jp = pps.tile([128, 512], f32, tag="proj", name="junk_ps")
                for _ in range(nmm):
                    nc.tensor.matmul(jp[:], junk_sb[:, 0:128], junk_sb[:],
                                     start=True, stop=True)

            # ---------------- fill queue with drain sentinels ----------------
            fill_q = []

            def fill(n=1):
                done = 0
                while fill_q and done < n:
                    item = fill_q[0]
                    if isinstance(item, str):
                        break  # sentinels are only crossed by drain_until
                    fill_q.pop(0)()
                    done += 1

            def drain_until(marker):
                while fill_q:
                    item = fill_q.pop(0)
                    if isinstance(item, str):
                        if item == marker:
                            return
                        continue
                    item()

            def queue_wave(sb, rp):
                """Queue seq-block-sb projections as <=0.5us closures.
                Layout: [q-t0] <A{sb}> [k-t0, v x4] <M{sb}> [q-t1, k-t1]
                <B{sb}>: run (0,sb) needs only q-t0 at its first chunk
                (k/v blocks of wave sb are first read at chunk 4*sb), so the
                A-drain block at the run boundary stays small.  The x DMA is
                issued immediately (it costs no PE time; the 4 x buffers
                make early issue hazard-free)."""
                nc.sync.dma_start(xT_sbs[sb][:],
                                  xT[:, :, sb * 512:(sb + 1) * 512])
                fill_q.extend(proj_qk_closures(qT_sb, wq_sb, 0, sb, rp))
                fill_q.append(f"A{sb}")
                fill_q.extend(proj_qk_closures(kT_sb, wk_sb, 0, sb, rp))
                for sc in range(4 * sb, 4 * sb + 4):
                    fill_q.extend(proj_v_closures(sc, rp))
                fill_q.append(f"M{sb}")
                fill_q.extend(proj_qk_closures(qT_sb, wq_sb, 1, sb, rp))
                fill_q.extend(proj_qk_closures(kT_sb, wk_sb, 1, sb, rp))
                fill_q.append(f"B{sb}")

            # ---------------- flat attention pipeline ----------------
            def attn_all(rp):
                chunks = []
                for qb in range(NQB):
                    for pair in range(2):
                        for c in range(4 * qb + 4):
                            chunks.append((pair, qb, c))
                n = len(chunks)
                o_ps_by_run = {}

                def emit_sc(i):
                    pair, qb, c = chunks[i]
                    if c == 0:
                        # run boundary: everything this run reads must have
                        # been emitted already (the scheduler leans heavily
                        # on emission order within each engine)
                        drain_until(f"A{qb}" if pair == 0 else f"B{qb}")
                        if qb > 0 and pair == 0 and qb + 1 < NQB:
                            queue_wave(qb + 1, rp)
                        o_ps_by_run[(pair, qb)] = [
                            ops.tile([128, 4, 65], f32, tag=f"o{h}",
                                     name=f"o_ps{rp}_{pair}_{qb}_{h}")
                            for h in range(2)]
                    elif c == 4 * qb and pair == 0 and qb > 0:
                        # wave-qb k/v blocks are first read from this chunk on
                        drain_until(f"M{qb}")
                    s = c - 4 * qb
                    lo = 0 if s < 0 else 128 * s
                    qlo = qb * 512
                    sc_t = scp.tile([128, 2, 512], f32, tag="sc",
                                    name=f"sc_{rp}_{pair}_{qb}_{c}")
                    for h in range(2):
                        nc.tensor.matmul(
                            sc_t[:, h, lo:],
                            kT_sb[h * 64:(h + 1) * 64, pair,
                                  c * 128:(c + 1) * 128],
                            qT_sb[h * 64:(h + 1) * 64, pair,
                                  qlo + lo:qlo + 512],
                            start=True, stop=True)
                    return sc_t

                def emit_post(i, sc_t):
                    pair, qb, c = chunks[i]
                    s = c - 4 * qb
                    lo = 0 if s < 0 else 128 * s
                    pt = ptp.tile([128, 2, 512], f16, tag="pt")
                    nc.scalar.activation(
                        pt[:, :, lo:], sc_t[:, :, lo:], EXP, scale=0.125)
                    if s >= 0:
                        nc.vector.tensor_tensor(
                            pt[:, :, lo:lo + 128],
                            pt[:, :, lo:lo + 128],
                            tri_sb[:].unsqueeze(1).broadcast_to([128, 2, 128]),
                            MUL)
                    return pt

                def emit_av(i, pt):
                    pair, qb, c = chunks[i]
                    s = c - 4 * qb
                    o_ps = o_ps_by_run[(pair, qb)]
                    last_run = (pair == 1 and qb == NQB - 1)
                    qlo = qb * 512
                    for j in range(max(0, s), 4):
                        for h in range(2):
                            nc.tensor.matmul(
                                o_ps[h][:, j, :],
                                pt[:, h, j * 128:(j + 1) * 128],
                                vx_sb[:, c, 2 * pair + h, :],
                                start=(c == 0), stop=(c == 4 * qb + j))
                        if last_run and c == 4 * qb + j:
                            # final run: flush each q-subchunk as soon as its
                            # accumulation stops, so the drain after the last
                            # exp is one 128x65 copy+DMA instead of the full
                            # two-head flush
                            if j == 0:
                                o_sb_last[0] = ost.tile(
                                    [128, 2, 4, 65], f32, tag="ost",
                                    name="o_sb_final")
                            o_sb = o_sb_last[0]
                            for h, eng in ((0, nc.gpsimd), (1, nc.vector)):
                                eng.tensor_copy(o_sb[:, h, j], o_ps[h][:, j, :])
                                nc.sync.dma_start(
                                    o[2 * pair + h,
                                      qlo + j * 128:qlo + (j + 1) * 128, :],
                                    o_sb[:, h, j])
                    if c == 4 * qb + 3 and not last_run:  # flush whole run
                        o_sb = ost.tile([128, 2, 4, 65], f32, tag="ost")
                        for h in range(2):
                            nc.gpsimd.tensor_copy(o_sb[:, h], o_ps[h][:])
                            nc.sync.dma_start(
                                o[2 * pair + h, qlo:qlo + 512, :]
                                .rearrange("(j p) d -> p j d", j=4),
                                o_sb[:, h])

                o_sb_last = [None]

                sc_t = emit_sc(0)
                for i in range(n):
                    pt = emit_post(i, sc_t)
                    if i + 1 < n:
                        sc_t = emit_sc(i + 1)
                    fill(2)
                    emit_av(i, pt)
                drain_until("nonexistent")  # safety: empty the queue

            if timing:
                dpool = cst.tile([1, 64], f32, tag="dumm", name="dumm")
                nc.sync.dma_start(dpool[:], dummy_in)
                nc.sync.dma_start(dummy_out, dpool[:])
            for rp in range(reps):
                # Wave 0, minimal critical path to the first exp: halved
                # DMAs so the first projection matmuls (kc 0..3) can start
                # while the second halves are still in flight.
                nc.sync.dma_start(xT_sbs[0][:, 0:4], xT[:, 0:4, 0:512])
                nc.sync.dma_start(wq_sb[:, 0:4], wq[:, 0:4])
                nc.sync.dma_start(wk_sb[:, 0:4], wk[:, 0:4])
                nc.sync.dma_start(xT_sbs[0][:, 4:8], xT[:, 4:8, 0:512])
                nc.sync.dma_start(wq_sb[:, 4:8], wq[:, 4:8])
                nc.sync.dma_start(wk_sb[:, 4:8], wk[:, 4:8])
                nc.sync.dma_start(cos_sb[:, 0:512], cosT[:, 0:512])
                nc.sync.dma_start(sin_sb[:, 0:512], sinT[:, 0:512])
                nc.sync.dma_start(tri_sb[:], tri)
                nc.sync.dma_start(wv_sb[:], wv[:])
                nc.sync.dma_start(vx_sb[:, :, :, 64], vones)
                nc.sync.dma_start(cos_sb[:, 512:], cosT[:, 512:])
                nc.sync.dma_start(sin_sb[:, 512:], sinT[:, 512:])
                pe_warmup(7)
                # pair-0 q/k inline (attention can then start); v chunks 0..3
                # go first in the fill queue, popped 2-per-chunk during run
                # (0,0) just in time for each chunk's AV.
                b = [None]
                proj_qk_mm(qT_sb, wq_sb, 0, 0, rp, list(range(NKC)), b)
                b = [None]
                proj_qk_mm(kT_sb, wk_sb, 0, 0, rp, list(range(NKC)), b)
                for sc in range(4):
                    fill_q.extend(proj_v_closures(sc, rp))
                # pair-1 q/k of wave 0 go through the fill queue (spliced into
                # attention run (0,0)); then wave 1.
                fill_q.append("A0")
                fill_q.extend(proj_qk_closures(qT_sb, wq_sb, 1, 0, rp))
                fill_q.extend(proj_qk_closures(kT_sb, wk_sb, 1, 0, rp))
                fill_q.append("B0")
                queue_wave(1, rp)
                attn_all(rp)

    nc.compile()
    _CACHE[key] = nc
    return nc


# --------------------------------------------------------------------------
# host-side sharding / unsharding
# --------------------------------------------------------------------------
def _make_in_maps(x, Wq, Wkv):
    x = np.asarray(x, np.float32)
    Wq = np.asarray(Wq, np.float32)
    Wkv = np.asarray(Wkv, np.float32)

    dp = _dperm()
    cos32, sin32 = _rope_tables()
    sign = np.where((np.arange(128) % 32) < 16, -1.0, 1.0)
    rows64 = np.concatenate([dp, dp])                       # 128 rows, 2 heads
    cosT = cos32[:, rows64 % 32].T.astype(np.float16)       # (128, S)
    sinT = (sin32[:, rows64 % 32].T * sign[:, None]).astype(np.float16)
    tri = (np.arange(128)[:, None] <= np.arange(128)[None, :]).astype(np.float16)

    # x[b].T is (D, S) = (NKC*128, S); partition-major: (128, NKC, S)
    xT_b = [np.ascontiguousarray(
                x[b].T.reshape(NKC, 128, S).transpose(1, 0, 2)
            ).astype(np.float16) for b in range(B)]

    in_maps = []
    for c in range(NCORES):
        b, g = divmod(c, 4)
        heads = [4 * g + hh for hh in range(4)]
        qrows = np.concatenate([h * 64 + dp for h in heads])
        krows = np.concatenate([h * 128 + 2 * dp for h in heads])
        vrows = np.concatenate([h * 128 + 2 * np.arange(64) + 1 for h in heads])

        def wmap(Wrows):  # (256, D) -> partition-major (128, NKC, 256) fp16
            wT = Wrows.T.reshape(NKC, 128, 256).transpose(1, 0, 2)
            return np.ascontiguousarray(wT).astype(np.float16)

        in_maps.append({
            "xT": xT_b[b],
            "wq": wmap(Wq[qrows, :]),
            "wk": wmap(Wkv[krows, :]),
            "wv": wmap(Wkv[vrows, :]),
            "cosT": cosT, "sinT": sinT, "tri": tri,
            "vones": np.ones((128, NSC, 4), np.float16),
        })
    return in_maps


def _assemble(results):
    out = np.empty((B, S, D), np.float32)
    for c in range(NCORES):
        b, g = divmod(c, 4)
        oc = results[c]["o"]                        # (4, S, 65)
        att = oc[:, :, :64] / oc[:, :, 64:65]       # (4, S, 64)
        for hh in range(4):
            head = 4 * g + hh
            out[b, :, head * 64:(head + 1) * 64] = att[hh]
    return out


def kernel(x, Wq, Wkv, mask=None):
    from concourse.bass_utils import run_bass_kernel_spmd

    nc = _build()
    in_maps = _make_in_maps(x, Wq, Wkv)
    res = run_bass_kernel_spmd(nc, in_maps, core_ids=list(range(NCORES)))
    return _assemble(res.results)
